# revision 1
# baseline (speedup 1.0000x reference)
"""Multi-head causal attention kernel for 8 Trainium2 NeuronCores.

Problem: B=2, T=4096, D=512, H=8 (DH=64) fp32 MHA with causal mask.

Sharding: 16 (b, h) pairs -> 2 head-pairs per core (core c: b = c//4,
heads 2*(c%4), 2*(c%4)+1). Each core:
  - projects q/k into feature-major (DH x T) layout and v into t-major
    (T x DH) layout directly from host-pre-transposed inputs,
  - runs causal flash-style attention per head: scoresT = kT.T-block @ qT
    (PE), exp on ScalarE (scale=1/8 folded in), per-block causal masks on
    DVE, AV.T + rowsum accumulated in PSUM via a ones-column in the
    stationary operand, normalization via reciprocal + partition
    broadcast,
  - applies the output projection for its 2 heads producing a partial
    (T, D) output.
Host sums the 4 partials per batch and adds the output bias.

The mask is verified host-side to be the causal tril; if it is not, a
numpy fallback computes the exact reference result.
"""

import os
import numpy as np

B, T, D, H = 2, 4096, 512, 8
DH = D // H          # 64
HPC = 2              # heads per core
NCORES = 8
QG = 512             # query-group width (matmul moving-operand size)
NQG = T // QG        # 8
NT = T // 128        # 32 key tiles
CCH = D // 128       # 4 contraction chunks for projections

LAST_EXEC_TIME_NS = None
LAST_RESULTS = None


def _build_module():
    import concourse.bacc as bacc
    import concourse.tile as tile
    from concourse import mybir
    from contextlib import ExitStack

    f32 = mybir.dt.float32
    bf16 = mybir.dt.bfloat16
    EXP = mybir.ActivationFunctionType.Exp

    nc = bacc.Bacc("TRN2", target_bir_lowering=False, debug=False)

    xqT = nc.dram_tensor("xqT", (D, T), f32, kind="ExternalInput")
    xkT = nc.dram_tensor("xkT", (D, T), f32, kind="ExternalInput")
    xvT = nc.dram_tensor("xvT", (D, T), f32, kind="ExternalInput")
    wq = nc.dram_tensor("wq", (D, HPC * DH), f32, kind="ExternalInput")
    wk = nc.dram_tensor("wk", (D, HPC * DH), f32, kind="ExternalInput")
    wv = nc.dram_tensor("wv", (D, HPC * DH), f32, kind="ExternalInput")
    woa = nc.dram_tensor("woa", (DH, D), f32, kind="ExternalInput")
    wob = nc.dram_tensor("wob", (DH, D), f32, kind="ExternalInput")
    bq2 = nc.dram_tensor("bq2", (HPC * DH, 1), f32, kind="ExternalInput")
    bk2 = nc.dram_tensor("bk2", (HPC * DH, 1), f32, kind="ExternalInput")
    bvr = nc.dram_tensor("bvr", (1, HPC * DH), f32, kind="ExternalInput")
    cmask = nc.dram_tensor("cmask", (128, 4, QG), f32, kind="ExternalInput")
    out_part = nc.dram_tensor("out_part", (T, D), f32, kind="ExternalOutput")

    with tile.TileContext(nc) as tc, ExitStack() as ctx:
        const = ctx.enter_context(tc.tile_pool(name="const", bufs=1))
        resid = ctx.enter_context(tc.tile_pool(name="resid", bufs=1))
        raws = ctx.enter_context(tc.tile_pool(name="raws", bufs=6))
        ppool = ctx.enter_context(tc.tile_pool(name="ppool", bufs=4))
        apool = ctx.enter_context(tc.tile_pool(name="apool", bufs=4))
        opool = ctx.enter_context(tc.tile_pool(name="opool", bufs=3))
        pscore = ctx.enter_context(tc.tile_pool(name="pscore", bufs=2, space="PSUM"))
        pmisc = ctx.enter_context(tc.tile_pool(name="pmisc", bufs=4, space="PSUM"))

        # ---- constants (cast fp32 -> bf16 in the DMA where needed) ----
        wq_sb = const.tile([128, CCH, HPC * DH], bf16)
        nc.gpsimd.dma_start(out=wq_sb, in_=wq[:].rearrange("(c p) m -> p c m", p=128))
        wk_sb = const.tile([128, CCH, HPC * DH], bf16)
        nc.gpsimd.dma_start(out=wk_sb, in_=wk[:].rearrange("(c p) m -> p c m", p=128))
        wv_sb = const.tile([128, CCH, HPC * DH], bf16)
        nc.gpsimd.dma_start(out=wv_sb, in_=wv[:].rearrange("(c p) m -> p c m", p=128))
        woa_sb = const.tile([DH, D], bf16)
        nc.gpsimd.dma_start(out=woa_sb, in_=woa[:])
        wob_sb = const.tile([DH, D], bf16)
        nc.gpsimd.dma_start(out=wob_sb, in_=wob[:])
        bq_sb = const.tile([HPC * DH, 1], f32)
        nc.gpsimd.dma_start(out=bq_sb, in_=bq2[:])
        bk_sb = const.tile([HPC * DH, 1], f32)
        nc.gpsimd.dma_start(out=bk_sb, in_=bk2[:])
        bvr_sb = const.tile([1, HPC * DH], bf16)
        nc.gpsimd.dma_start(out=bvr_sb, in_=bvr[:])
        cmask_sb = const.tile([128, 4, QG], bf16)
        nc.gpsimd.dma_start(out=cmask_sb, in_=cmask[:])
        ones1_sb = const.tile([1, 128], bf16)
        nc.vector.memset(ones1_sb, 1.0)

        # ---- residents ----
        qT_sb = resid.tile([HPC * DH, T], bf16)   # feature-major q, 2 heads
        kT_sb = resid.tile([HPC * DH, T], bf16)   # feature-major k, 2 heads
        # t-major v, per key-tile: [vA(64) | 1] [vB(64) | 1]
        v_sb = resid.tile([128, NT, HPC, DH + 1], bf16)
        nc.vector.memset(v_sb[:, :, :, DH], 1.0)

        # ---- phase 1: projections ----
        for src, wsb, bias_sb, dst in (
            (xqT, wq_sb, bq_sb, qT_sb),
            (xkT, wk_sb, bk_sb, kT_sb),
        ):
            for tb in range(NQG):
                ps = pmisc.tile([128, QG], f32, tag="pm", name="ps_proj")
                for cc in range(CCH):
                    raw = raws.tile([128, QG], bf16, tag="raw", name="raw")
                    nc.gpsimd.dma_start(
                        out=raw,
                        in_=src[cc * 128:(cc + 1) * 128, tb * QG:(tb + 1) * QG],
                    )
                    nc.tensor.matmul(
                        ps, wsb[:, cc, :], raw,
                        start=(cc == 0), stop=(cc == CCH - 1),
                    )
                nc.vector.tensor_scalar_add(
                    dst[:, tb * QG:(tb + 1) * QG], ps, bias_sb
                )

        for tb in range(NQG):
            raw4 = []
            for cc in range(CCH):
                raw = raws.tile([128, QG], bf16, tag="raw", name="raw")
                nc.gpsimd.dma_start(
                    out=raw,
                    in_=xvT[cc * 128:(cc + 1) * 128, tb * QG:(tb + 1) * QG],
                )
                raw4.append(raw)
            for j in range(QG // 128):
                tt = tb * 4 + j
                ps = pmisc.tile([128, HPC * DH], f32, tag="pm", name="ps_v")
                for cc in range(CCH):
                    nc.tensor.matmul(
                        ps, raw4[cc][:, j * 128:(j + 1) * 128], wv_sb[:, cc, :],
                        start=(cc == 0), stop=False, skip_group_check=True,
                    )
                # bias: out[t, d] += 1 * bv[d]
                nc.tensor.matmul(
                    ps, ones1_sb, bvr_sb,
                    start=False, stop=True, skip_group_check=True,
                )
                nc.vector.tensor_copy(
                    v_sb[:, tt, :, 0:DH],
                    ps.rearrange("p (h d) -> p h d", h=HPC),
                )

        # ---- phase 2: attention + output projection ----
        for g in range(NQG):
            q0 = g * QG
            nkb = 4 * g + 4
            av_ps = []
            for h in range(HPC):
                av = pmisc.tile([DH + 1, QG], f32, tag="pm", name="av_ps")
                av_ps.append(av)
            for pair in range(nkb // 2):
                s_ps = []
                p_t = []
                for h in range(HPC):
                    s = pscore.tile([128, 2, QG], f32, tag="sc", name="s_ps")
                    s_ps.append(s)
                for i in range(2):
                    kb = pair * 2 + i
                    for h in range(HPC):
                        nc.tensor.matmul(
                            s_ps[h][:, i, :],
                            kT_sb[h * DH:(h + 1) * DH, kb * 128:(kb + 1) * 128],
                            qT_sb[h * DH:(h + 1) * DH, q0:q0 + QG],
                            start=True, stop=True,
                        )
                for h in range(HPC):
                    p = ppool.tile([128, 2, QG], bf16, tag="p", name="p_t")
                    nc.scalar.activation(p, s_ps[h], EXP, scale=0.125)
                    p_t.append(p)
                for i in range(2):
                    kb = pair * 2 + i
                    jj = kb - 4 * g
                    if jj >= 0:
                        for h in range(HPC):
                            nc.vector.tensor_mul(
                                p_t[h][:, i, :], p_t[h][:, i, :], cmask_sb[:, jj, :]
                            )
                for i in range(2):
                    kb = pair * 2 + i
                    for h in range(HPC):
                        nc.tensor.matmul(
                            av_ps[h], v_sb[:, kb, h, :], p_t[h][:, i, :],
                            start=(kb == 0), stop=(kb == nkb - 1),
                            skip_group_check=True,
                        )
            attn = []
            for h in range(HPC):
                rec = apool.tile([1, QG], f32, tag="rec", name="rec")
                nc.vector.reciprocal(rec, av_ps[h][DH:DH + 1, :])
                rb = apool.tile([DH, QG], f32, tag="rb", name="rb")
                nc.gpsimd.partition_broadcast(rb, rec)
                at = apool.tile([DH, QG], bf16, tag="at", name="at")
                nc.vector.tensor_mul(at, av_ps[h][0:DH, :], rb)
                attn.append(at)
            for j in range(QG // 128):
                o_ps = pmisc.tile([128, D], f32, tag="pm", name="o_ps")
                nc.tensor.matmul(
                    o_ps, attn[0][:, j * 128:(j + 1) * 128], woa_sb,
                    start=True, stop=False, skip_group_check=True,
                )
                nc.tensor.matmul(
                    o_ps, attn[1][:, j * 128:(j + 1) * 128], wob_sb,
                    start=False, stop=True, skip_group_check=True,
                )
                ot = opool.tile([128, D], f32, tag="ot", name="ot")
                nc.vector.tensor_copy(ot, o_ps)
                nc.sync.dma_start(
                    out=out_part[q0 + j * 128:q0 + (j + 1) * 128, :], in_=ot
                )

    nc.compile()
    return nc


def _numpy_reference(query, key, value, mask, Wq, bq, Wk, bk, Wv, bv, Wo, bo):
    def split_heads(x):
        b, t, d = x.shape
        return x.reshape(b, t, H, DH).transpose(0, 2, 1, 3)

    q = split_heads(query @ Wq.T + bq)
    k = split_heads(key @ Wk.T + bk)
    v = split_heads(value @ Wv.T + bv)
    scale = 1.0 / np.sqrt(np.float32(DH))
    out = np.empty((B, H, T, DH), np.float32)
    for b in range(B):
        for h in range(H):
            s = (q[b, h] @ k[b, h].T) * scale
            s = np.where(mask[b] == 0, -np.inf, s)
            s = s - s.max(axis=-1, keepdims=True)
            p = np.exp(s)
            p /= p.sum(axis=-1, keepdims=True)
            out[b, h] = p @ v[b, h]
    out = out.transpose(0, 2, 1, 3).reshape(B, T, D)
    return out @ Wo.T + bo


def kernel(query, key, value, mask, Wq, bq, Wk, bk, Wv, bv, Wo, bo):
    global LAST_EXEC_TIME_NS, LAST_RESULTS
    query = np.asarray(query, np.float32)
    key = np.asarray(key, np.float32)
    value = np.asarray(value, np.float32)
    mask = np.asarray(mask)
    Wq, bq = np.asarray(Wq, np.float32), np.asarray(bq, np.float32)
    Wk, bk = np.asarray(Wk, np.float32), np.asarray(bk, np.float32)
    Wv, bv = np.asarray(Wv, np.float32), np.asarray(bv, np.float32)
    Wo, bo = np.asarray(Wo, np.float32), np.asarray(bo, np.float32)

    tril = np.tril(np.ones((T, T), mask.dtype))
    causal = all(np.array_equal(mask[b], tril) for b in range(B))
    if not causal:
        return _numpy_reference(
            query, key, value, mask, Wq, bq, Wk, bk, Wv, bv, Wo, bo
        ).astype(np.float32)

    r = np.arange(128, dtype=np.int64)[:, None]
    c = np.arange(QG, dtype=np.int64)[None, :]
    cmask = np.stack(
        [(c >= 128 * j + r).astype(np.float32) for j in range(4)], axis=1
    )  # (128, 4, QG)

    in_maps = []
    for core in range(NCORES):
        b = core // 4
        h0 = (core % 4) * HPC
        sl = slice(h0 * DH, (h0 + HPC) * DH)
        in_maps.append({
            "xqT": np.ascontiguousarray(query[b].T),
            "xkT": np.ascontiguousarray(key[b].T),
            "xvT": np.ascontiguousarray(value[b].T),
            "wq": np.ascontiguousarray(Wq[sl, :].T),
            "wk": np.ascontiguousarray(Wk[sl, :].T),
            "wv": np.ascontiguousarray(Wv[sl, :].T),
            "woa": np.ascontiguousarray(Wo[:, h0 * DH:(h0 + 1) * DH].T),
            "wob": np.ascontiguousarray(Wo[:, (h0 + 1) * DH:(h0 + 2) * DH].T),
            "bq2": np.ascontiguousarray(bq[sl].reshape(HPC * DH, 1)),
            "bk2": np.ascontiguousarray(bk[sl].reshape(HPC * DH, 1)),
            "bvr": np.ascontiguousarray(bv[sl].reshape(1, HPC * DH)),
            "cmask": cmask,
        })

    nc = _build_module()
    from concourse import bass_utils

    trace = os.environ.get("KERNEL_TRACE", "0") == "1"
    res = bass_utils.run_bass_kernel_spmd(
        nc, in_maps, core_ids=list(range(NCORES)), trace=trace
    )
    LAST_RESULTS = res
    LAST_EXEC_TIME_NS = res.exec_time_ns

    out = np.zeros((B, T, D), np.float32)
    for core in range(NCORES):
        out[core // 4] += np.asarray(res.results[core]["out_part"], np.float32)
    out += bo[None, None, :]
    return out


# revision 6
# speedup vs baseline: 1.3070x; 1.3070x over previous
"""Multi-head causal attention kernel for 8 Trainium2 NeuronCores.

Problem: B=2, T=4096, D=512, H=8 (DH=64) fp32 MHA with causal mask.

Sharding: 16 (b, h) pairs -> 2 heads per core (core c: b = c//4, heads
2*(c%4), 2*(c%4)+1). Each core projects q/k into feature-major (DH x T)
layout and v into t-major (T x DH) layout from host-pre-transposed
inputs, runs causal flash-style attention per head (scoresT on PE, exp
on ScalarE with the 1/sqrt(dh) scale folded in, per-block causal masks
on DVE, AV.T + rowsum accumulated in PSUM via a ones-column in the
stationary operand), normalizes via reciprocal + partition broadcast,
and applies the output projection for its 2 heads producing a partial
(T, D) output. The host sums the 4 partials per batch and adds the
output bias.

The projection work for t-block tb is interleaved with the attention
work for query-group g=tb so the PE stays dense (and HAM-warm) while
the raw input stream DMAs in; scores/exp/AV/normalize are software-
pipelined one step apart for the same reason.

The mask is verified host-side to be the causal tril; if not, a numpy
fallback computes the exact reference result.
"""

import os
import numpy as np

B, T, D, H = 2, 4096, 512, 8
DH = D // H          # 64
HPC = 2              # heads per core
NCORES = 8
QG = 512             # query-group width (matmul moving-operand size)
NQG = T // QG        # 8
NT = T // 128        # 32 key tiles
CCH = D // 128       # 4 contraction chunks for projections

# wpack column layout (all cast to bf16 on load): wq | wk | wv | wo | cmask
# wo region is 1024 cols with data only in partitions 0..63 ([woA | woB])
# so both O-proj operands sit at partition base 0.
WQ0, WK0, WV0 = 0, 512, 1024
WO0 = 1536
CM0 = 2560
WPACK_COLS = 4608

LAST_EXEC_TIME_NS = None
LAST_RESULTS = None


def _build_module():
    import concourse.bacc as bacc
    import concourse.tile as tile
    from concourse import mybir
    from contextlib import ExitStack

    f32 = mybir.dt.float32
    bf16 = mybir.dt.bfloat16
    EXP = mybir.ActivationFunctionType.Exp

    nc = bacc.Bacc("TRN2", target_bir_lowering=False, debug=False)

    xqT = nc.dram_tensor("xqT", (D, T), f32, kind="ExternalInput")
    xkT = nc.dram_tensor("xkT", (D, T), f32, kind="ExternalInput")
    xvT = nc.dram_tensor("xvT", (D, T), f32, kind="ExternalInput")
    wpack = nc.dram_tensor("wpack", (128, WPACK_COLS), f32, kind="ExternalInput")
    bq2 = nc.dram_tensor("bq2", (HPC * DH, 1), f32, kind="ExternalInput")
    bk2 = nc.dram_tensor("bk2", (HPC * DH, 1), f32, kind="ExternalInput")
    bvr = nc.dram_tensor("bvr", (1, HPC * DH), f32, kind="ExternalInput")
    out_part = nc.dram_tensor("out_part", (T, D), f32, kind="ExternalOutput")

    with tile.TileContext(nc) as tc, ExitStack() as ctx:
        const = ctx.enter_context(tc.tile_pool(name="const", bufs=1))
        resid = ctx.enter_context(tc.tile_pool(name="resid", bufs=1))
        raws = ctx.enter_context(tc.tile_pool(name="raws", bufs=8))
        ppool = ctx.enter_context(tc.tile_pool(name="ppool", bufs=4))
        apool = ctx.enter_context(tc.tile_pool(name="apool", bufs=4))
        opool = ctx.enter_context(tc.tile_pool(name="opool", bufs=3))
        pscore = ctx.enter_context(tc.tile_pool(name="pscore", bufs=2, space="PSUM"))
        pmisc = ctx.enter_context(tc.tile_pool(name="pmisc", bufs=4, space="PSUM"))

        # ---- constants: one packed cast-DMA + three tiny bias DMAs ----
        wp_sb = const.tile([128, WPACK_COLS], bf16)
        nc.gpsimd.dma_start(out=wp_sb, in_=wpack[:])
        wq_sb = wp_sb[:, WQ0:WQ0 + 512].rearrange("p (c m) -> p c m", c=CCH)
        wk_sb = wp_sb[:, WK0:WK0 + 512].rearrange("p (c m) -> p c m", c=CCH)
        wv_sb = wp_sb[:, WV0:WV0 + 512].rearrange("p (c m) -> p c m", c=CCH)
        woa_sb = wp_sb[0:DH, WO0:WO0 + 512]                  # [64, 512]
        wob_sb = wp_sb[0:DH, WO0 + 512:WO0 + 1024]           # [64, 512]
        cm_sb = wp_sb[:, CM0:CM0 + 2048].rearrange("p (j q) -> p j q", j=4)

        bq_sb = const.tile([HPC * DH, 1], f32)
        nc.sync.dma_start(out=bq_sb, in_=bq2[:])
        bk_sb = const.tile([HPC * DH, 1], f32)
        nc.sync.dma_start(out=bk_sb, in_=bk2[:])
        bvr_sb = const.tile([1, HPC * DH], bf16)
        nc.gpsimd.dma_start(out=bvr_sb, in_=bvr[:])
        ones1_sb = const.tile([1, 128], bf16)
        nc.vector.memset(ones1_sb, 1.0)

        # ---- residents ----
        qT_sb = resid.tile([HPC * DH, T], bf16)   # feature-major q, 2 heads
        kT_sb = resid.tile([HPC * DH, T], bf16)   # feature-major k, 2 heads
        # t-major v, per key-tile: [vA(64) | 1] [vB(64) | 1]
        v_sb = resid.tile([128, NT, HPC, DH + 1], bf16)
        nc.vector.memset(v_sb[:, :, :, DH], 1.0)

        # ---- emission helpers -------------------------------------------
        def emit_proj_qk(tb):
            for src, wsb, bias_sb, dst in (
                (xkT, wk_sb, bk_sb, kT_sb),
                (xqT, wq_sb, bq_sb, qT_sb),
            ):
                ps = pmisc.tile([128, QG], f32, tag="pm", name="ps_proj")
                for cc in range(CCH):
                    raw = raws.tile([128, QG], bf16, tag="raw", name="raw")
                    nc.gpsimd.dma_start(
                        out=raw,
                        in_=src[cc * 128:(cc + 1) * 128, tb * QG:(tb + 1) * QG],
                    )
                    nc.tensor.matmul(
                        ps, wsb[:, cc, :], raw,
                        start=(cc == 0), stop=(cc == CCH - 1),
                    )
                nc.vector.tensor_scalar_add(
                    dst[:, tb * QG:(tb + 1) * QG], ps, bias_sb
                )

        def emit_proj_v(tb):
            raw4 = []
            for cc in range(CCH):
                raw = raws.tile([128, QG], bf16, tag="raw", name="raw")
                nc.gpsimd.dma_start(
                    out=raw,
                    in_=xvT[cc * 128:(cc + 1) * 128, tb * QG:(tb + 1) * QG],
                )
                raw4.append(raw)
            for j in range(QG // 128):
                tt = tb * 4 + j
                ps = pmisc.tile([128, HPC * DH], f32, tag="pm", name="ps_v")
                for cc in range(CCH):
                    nc.tensor.matmul(
                        ps, raw4[cc][:, j * 128:(j + 1) * 128], wv_sb[:, cc, :],
                        start=(cc == 0), stop=False, skip_group_check=True,
                    )
                nc.tensor.matmul(     # bias: out[t, d] += 1 * bv[d]
                    ps, ones1_sb, bvr_sb,
                    start=False, stop=True, skip_group_check=True,
                )
                nc.vector.tensor_copy(
                    v_sb[:, tt, :, 0:DH],
                    ps.rearrange("p (h d) -> p h d", h=HPC),
                )

        def emit_scores(g, pair):
            q0 = g * QG
            s_ps, p_t = [], []
            for h in range(HPC):
                s = pscore.tile([128, 2, QG], f32, tag="sc", name="s_ps")
                s_ps.append(s)
            for i in range(2):
                kb = pair * 2 + i
                for h in range(HPC):
                    nc.tensor.matmul(
                        s_ps[h][:, i, :],
                        kT_sb[h * DH:(h + 1) * DH, kb * 128:(kb + 1) * 128],
                        qT_sb[h * DH:(h + 1) * DH, q0:q0 + QG],
                        start=True, stop=True,
                        tile_position=(h * DH, 0),
                    )
            for h in range(HPC):
                p = ppool.tile([128, 2, QG], bf16, tag="p", name="p_t")
                nc.scalar.activation(p, s_ps[h], EXP, scale=0.125)
                p_t.append(p)
            for i in range(2):
                jj = pair * 2 + i - 4 * g
                if jj >= 0:
                    for h in range(HPC):
                        nc.vector.tensor_mul(
                            p_t[h][:, i, :], p_t[h][:, i, :], cm_sb[:, jj, :]
                        )
            return p_t

        def make_av(g, pair, p_t, av_ps):
            nkb = 4 * g + 4

            def emit_av():
                for i in range(2):
                    kb = pair * 2 + i
                    for h in range(HPC):
                        nc.tensor.matmul(
                            av_ps[h], v_sb[:, kb, h, :], p_t[h][:, i, :],
                            start=(kb == 0), stop=(kb == nkb - 1),
                            skip_group_check=True,
                        )
            return emit_av

        def make_norm(g, av_ps):
            def emit_norm():
                attn = []
                for h in range(HPC):
                    rec = apool.tile([1, QG], f32, tag="rec", name="rec")
                    nc.vector.reciprocal(rec, av_ps[h][DH:DH + 1, :])
                    rb = apool.tile([DH, QG], f32, tag="rb", name="rb")
                    nc.gpsimd.partition_broadcast(rb, rec)
                    at = apool.tile([DH, QG], bf16, tag="at", name="at")
                    nc.vector.tensor_mul(at, av_ps[h][0:DH, :], rb)
                    attn.append(at)
                return attn
            return emit_norm

        def make_oproj(g, attn):
            q0 = g * QG

            def emit_oproj():
                for j in range(QG // 128):
                    o_ps = pmisc.tile([128, D], f32, tag="pm", name="o_ps")
                    nc.tensor.matmul(
                        o_ps, attn[0][:, j * 128:(j + 1) * 128], woa_sb,
                        start=True, stop=False, skip_group_check=True,
                    )
                    nc.tensor.matmul(
                        o_ps, attn[1][:, j * 128:(j + 1) * 128], wob_sb,
                        start=False, stop=True, skip_group_check=True,
                    )
                    ot = opool.tile([128, D], f32, tag="ot", name="ot")
                    nc.vector.tensor_copy(ot, o_ps)
                    nc.sync.dma_start(
                        out=out_part[q0 + j * 128:q0 + (j + 1) * 128, :], in_=ot
                    )
            return emit_oproj

        # ---- main interleaved loop --------------------------------------
        # Per g: project t-block g (k/q/v), then attention pairs for query
        # group g. AV lags scores by one pair; normalize+oproj of group g-1
        # are flushed inside group g's first two pair iterations.
        prev_av = None        # AV emission for the previous (g, pair)
        pend_norm = None      # normalize emission for the previous group
        pend_oproj_mk = None  # (g-1, attn) -> oproj emission
        for g in range(NQG):
            emit_proj_qk(g)
            emit_proj_v(g)
            av_ps = [
                pmisc.tile([DH + 1, QG], f32, tag="pm", name="av_ps")
                for _ in range(HPC)
            ]
            for pair in range(2 * g + 2):
                p_t = emit_scores(g, pair)
                if prev_av is not None:
                    prev_av()
                if pend_norm is not None:
                    attn_prev = pend_norm()
                    pend_oproj_mk = make_oproj(g - 1, attn_prev)
                    pend_norm = None
                elif pend_oproj_mk is not None:
                    pend_oproj_mk()
                    pend_oproj_mk = None
                prev_av = make_av(g, pair, p_t, av_ps)
            prev_av()
            prev_av = None
            pend_norm = make_norm(g, av_ps)
        attn_last = pend_norm()
        make_oproj(NQG - 1, attn_last)()

    nc.compile()
    return nc


def _numpy_reference(query, key, value, mask, Wq, bq, Wk, bk, Wv, bv, Wo, bo):
    def split_heads(x):
        b, t, d = x.shape
        return x.reshape(b, t, H, DH).transpose(0, 2, 1, 3)

    q = split_heads(query @ Wq.T + bq)
    k = split_heads(key @ Wk.T + bk)
    v = split_heads(value @ Wv.T + bv)
    scale = 1.0 / np.sqrt(np.float32(DH))
    out = np.empty((B, H, T, DH), np.float32)
    for b in range(B):
        for h in range(H):
            s = (q[b, h] @ k[b, h].T) * scale
            s = np.where(mask[b] == 0, -np.inf, s)
            s = s - s.max(axis=-1, keepdims=True)
            p = np.exp(s)
            p /= p.sum(axis=-1, keepdims=True)
            out[b, h] = p @ v[b, h]
    out = out.transpose(0, 2, 1, 3).reshape(B, T, D)
    return out @ Wo.T + bo


def kernel(query, key, value, mask, Wq, bq, Wk, bk, Wv, bv, Wo, bo):
    global LAST_EXEC_TIME_NS, LAST_RESULTS
    query = np.asarray(query, np.float32)
    key = np.asarray(key, np.float32)
    value = np.asarray(value, np.float32)
    mask = np.asarray(mask)
    Wq, bq = np.asarray(Wq, np.float32), np.asarray(bq, np.float32)
    Wk, bk = np.asarray(Wk, np.float32), np.asarray(bk, np.float32)
    Wv, bv = np.asarray(Wv, np.float32), np.asarray(bv, np.float32)
    Wo, bo = np.asarray(Wo, np.float32), np.asarray(bo, np.float32)

    tril = np.tril(np.ones((T, T), mask.dtype))
    causal = all(np.array_equal(mask[b], tril) for b in range(B))
    if not causal:
        return _numpy_reference(
            query, key, value, mask, Wq, bq, Wk, bk, Wv, bv, Wo, bo
        ).astype(np.float32)

    r = np.arange(128, dtype=np.int64)[:, None]
    c = np.arange(QG, dtype=np.int64)[None, :]
    cmask = np.stack(
        [(c >= 128 * j + r).astype(np.float32) for j in range(4)], axis=1
    )  # (128, 4, QG)

    in_maps = []
    for core in range(NCORES):
        b = core // 4
        h0 = (core % 4) * HPC
        sl = slice(h0 * DH, (h0 + HPC) * DH)
        # wpack: wq|wk|wv (each [128, 4, 128] -> 512 cols), wo stacked
        # rows (A above B) [128, 512], cmask [128, 4, 512] -> 2048 cols.
        wq_r = np.ascontiguousarray(Wq[sl, :].T).reshape(CCH, 128, 128).transpose(1, 0, 2).reshape(128, 512)
        wk_r = np.ascontiguousarray(Wk[sl, :].T).reshape(CCH, 128, 128).transpose(1, 0, 2).reshape(128, 512)
        wv_r = np.ascontiguousarray(Wv[sl, :].T).reshape(CCH, 128, 128).transpose(1, 0, 2).reshape(128, 512)
        wo_r = np.zeros((128, 1024), np.float32)
        wo_r[0:DH, 0:512] = Wo[:, h0 * DH:(h0 + 1) * DH].T
        wo_r[0:DH, 512:1024] = Wo[:, (h0 + 1) * DH:(h0 + 2) * DH].T
        wpk = np.concatenate(
            [wq_r, wk_r, wv_r, wo_r, cmask.reshape(128, 2048)], axis=1
        ).astype(np.float32)
        in_maps.append({
            "xqT": np.ascontiguousarray(query[b].T),
            "xkT": np.ascontiguousarray(key[b].T),
            "xvT": np.ascontiguousarray(value[b].T),
            "wpack": np.ascontiguousarray(wpk),
            "bq2": np.ascontiguousarray(bq[sl].reshape(HPC * DH, 1)),
            "bk2": np.ascontiguousarray(bk[sl].reshape(HPC * DH, 1)),
            "bvr": np.ascontiguousarray(bv[sl].reshape(1, HPC * DH)),
        })

    nc = _build_module()
    from concourse import bass_utils

    trace = os.environ.get("KERNEL_TRACE", "0") == "1"
    res = bass_utils.run_bass_kernel_spmd(
        nc, in_maps, core_ids=list(range(NCORES)), trace=trace
    )
    LAST_RESULTS = res
    LAST_EXEC_TIME_NS = res.exec_time_ns

    out = np.zeros((B, T, D), np.float32)
    for core in range(NCORES):
        out[core // 4] += np.asarray(res.results[core]["out_part"], np.float32)
    out += bo[None, None, :]
    return out


# revision 14
# speedup vs baseline: 1.5213x; 1.1640x over previous
"""Multi-head causal attention kernel for 8 Trainium2 NeuronCores.

Problem: B=2, T=4096, D=512, H=8 (DH=64) fp32 MHA with causal mask.

Sharding: 16 (b, h) pairs -> 2 heads per core (core c: b = c//4, heads
2*(c%4), 2*(c%4)+1). Each core projects q/k into feature-major (DH x T)
layout and v into t-major (T x DH) layout from host-pre-transposed
inputs, runs causal flash-style attention per head (scoresT on PE, exp
on ScalarE with the 1/sqrt(dh) scale folded in, per-block causal masks
on DVE, AV.T + rowsum accumulated in PSUM via a ones-column in the
stationary operand), normalizes via reciprocal + partition broadcast,
and applies the output projection for its 2 heads producing a partial
(T, D) output. The host sums the 4 partials per batch and adds the
output bias.

The projection work for t-block tb is interleaved with the attention
work for query-group g=tb so the PE stays dense (and HAM-warm) while
the raw input stream DMAs in; scores/exp/AV/normalize are software-
pipelined one step apart for the same reason.

The mask is verified host-side to be the causal tril; if not, a numpy
fallback computes the exact reference result.
"""

import os
import numpy as np

B, T, D, H = 2, 4096, 512, 8
DH = D // H          # 64
HPC = 2              # heads per core
NCORES = 8
QG = 512             # query-group width (matmul moving-operand size)
NQG = T // QG        # 8
NT = T // 128        # 32 key tiles
CCH = D // 128       # 4 contraction chunks for projections

# wpack column layout (all cast to bf16 on load): wq | wk | wv | wo.
# wo region is 1024 cols with data only in partitions 0..63 ([woA | woB])
# so both O-proj operands sit at partition base 0. The causal-mask
# patterns load separately so they don't delay the first projections.
WQ0, WK0, WV0 = 0, 512, 1024
WO0 = 1536
WPACK_COLS = 2560

LAST_EXEC_TIME_NS = None
LAST_RESULTS = None


def _build_module():
    import concourse.bacc as bacc
    import concourse.tile as tile
    from concourse import mybir
    from contextlib import ExitStack

    f32 = mybir.dt.float32
    bf16 = mybir.dt.bfloat16
    EXP = mybir.ActivationFunctionType.Exp

    nc = bacc.Bacc("TRN2", target_bir_lowering=False, debug=False)

    xqT = nc.dram_tensor("xqT", (D, T), f32, kind="ExternalInput")
    xkT = nc.dram_tensor("xkT", (D, T), f32, kind="ExternalInput")
    xvT = nc.dram_tensor("xvT", (D, T), f32, kind="ExternalInput")
    wpack = nc.dram_tensor("wpack", (128, WPACK_COLS), f32, kind="ExternalInput")
    cmdram = nc.dram_tensor("cmdram", (128, 4, QG), f32, kind="ExternalInput")
    bq2 = nc.dram_tensor("bq2", (HPC * DH, 1), f32, kind="ExternalInput")
    bk2 = nc.dram_tensor("bk2", (HPC * DH, 1), f32, kind="ExternalInput")
    bvr = nc.dram_tensor("bvr", (1, HPC * DH), f32, kind="ExternalInput")
    out_part = nc.dram_tensor("out_part", (T, D), f32, kind="ExternalOutput")

    with tile.TileContext(nc) as tc, ExitStack() as ctx:
        const = ctx.enter_context(tc.tile_pool(name="const", bufs=1))
        resid = ctx.enter_context(tc.tile_pool(name="resid", bufs=1))
        raws = ctx.enter_context(tc.tile_pool(name="raws", bufs=16))
        ppool = ctx.enter_context(tc.tile_pool(name="ppool", bufs=4))
        apool = ctx.enter_context(tc.tile_pool(name="apool", bufs=4))
        opool = ctx.enter_context(tc.tile_pool(name="opool", bufs=3))
        pscore = ctx.enter_context(tc.tile_pool(name="pscore", bufs=2, space="PSUM"))
        pmisc = ctx.enter_context(tc.tile_pool(name="pmisc", bufs=4, space="PSUM"))

        # ---- constants: weights first, mask patterns + biases after the
        # ---- first raw-input block so the first matmuls start early ----
        wp_sb = const.tile([128, WPACK_COLS], bf16)
        nc.gpsimd.dma_start(out=wp_sb, in_=wpack[:])
        wq_sb = wp_sb[:, WQ0:WQ0 + 512].rearrange("p (c m) -> p c m", c=CCH)
        wk_sb = wp_sb[:, WK0:WK0 + 512].rearrange("p (c m) -> p c m", c=CCH)
        wv_sb = wp_sb[:, WV0:WV0 + 512].rearrange("p (c m) -> p c m", c=CCH)
        woa_sb = wp_sb[0:DH, WO0:WO0 + 512]                  # [64, 512]
        wob_sb = wp_sb[0:DH, WO0 + 512:WO0 + 1024]           # [64, 512]

        # ---- residents ----
        qT_sb = resid.tile([HPC * DH, T], bf16)   # feature-major q, 2 heads
        kT_sb = resid.tile([HPC * DH, T], bf16)   # feature-major k, 2 heads
        # t-major v, per key-tile: [vA(64) | 1] [vB(64) | 1]
        v_sb = resid.tile([128, NT, HPC, DH + 1], bf16)
        nc.vector.memset(v_sb[:, :, :, DH], 1.0)

        # ---- emission helpers -------------------------------------------
        def emit_dma_block(tb):
            """Issue the 12 raw-input cast-DMAs for t-block tb."""
            tiles = {}
            for key, src in (("k", xkT), ("q", xqT), ("v", xvT)):
                for cc in range(CCH):
                    raw = raws.tile([128, QG], bf16, tag="raw", name="raw")
                    nc.gpsimd.dma_start(
                        out=raw,
                        in_=src[cc * 128:(cc + 1) * 128, tb * QG:(tb + 1) * QG],
                    )
                    tiles[key, cc] = raw
            return tiles

        def emit_proj(tb, rawt):
            for key, wsb, bias_sb, dst in (
                ("k", wk_sb, bk_sb, kT_sb),
                ("q", wq_sb, bq_sb, qT_sb),
            ):
                ps = pmisc.tile([128, QG], f32, tag="pm", name="ps_proj")
                for cc in range(CCH):
                    nc.tensor.matmul(
                        ps, wsb[:, cc, :], rawt[key, cc],
                        start=(cc == 0), stop=(cc == CCH - 1),
                    )
                nc.vector.tensor_scalar_add(
                    dst[:, tb * QG:(tb + 1) * QG], ps, bias_sb
                )
            for j in range(QG // 128):
                tt = tb * 4 + j
                ps = pmisc.tile([128, HPC * DH], f32, tag="pm", name="ps_v")
                for cc in range(CCH):
                    nc.tensor.matmul(
                        ps, rawt["v", cc][:, j * 128:(j + 1) * 128], wv_sb[:, cc, :],
                        start=(cc == 0), stop=False, skip_group_check=True,
                    )
                nc.tensor.matmul(     # bias: out[t, d] += 1 * bv[d]
                    ps, ones1_sb, bvr_sb,
                    start=False, stop=True, skip_group_check=True,
                )
                nc.vector.tensor_copy(
                    v_sb[:, tt, :, 0:DH],
                    ps.rearrange("p (h d) -> p h d", h=HPC),
                )

        def emit_scores(g, pair):
            # Boundary key-blocks (kb >= 4g) only attend to query columns
            # >= 128*jj within the group; restrict work to those columns.
            q0 = g * QG
            s_ps, p_t = [], []
            co = [max(0, (pair * 2 + i - 4 * g) * 128) for i in range(2)]
            for h in range(HPC):
                s = pscore.tile([128, 2, QG], f32, tag="sc", name="s_ps")
                s_ps.append(s)
            for i in range(2):
                kb = pair * 2 + i
                for h in range(HPC):
                    nc.tensor.matmul(
                        s_ps[h][:, i, co[i]:QG],
                        kT_sb[h * DH:(h + 1) * DH, kb * 128:(kb + 1) * 128],
                        qT_sb[h * DH:(h + 1) * DH, q0 + co[i]:q0 + QG],
                        start=True, stop=True,
                        tile_position=(h * DH, 0),
                    )
            for h in range(HPC):
                p = ppool.tile([128, 2, QG], bf16, tag="p", name="p_t")
                if co[0] == 0 and co[1] <= 128:
                    nc.scalar.activation(p, s_ps[h], EXP, scale=0.125)
                else:
                    for i in range(2):
                        nc.scalar.activation(
                            p[:, i, co[i]:QG], s_ps[h][:, i, co[i]:QG],
                            EXP, scale=0.125,
                        )
                p_t.append(p)
            for i in range(2):
                jj = pair * 2 + i - 4 * g
                if jj >= 0:
                    for h in range(HPC):
                        nc.vector.tensor_mul(
                            p_t[h][:, i, co[i]:QG], p_t[h][:, i, co[i]:QG],
                            cm_sb[:, jj, co[i]:QG],
                        )
            return p_t, co

        def make_av(g, pair, p_t, co, av_ps):
            nkb = 4 * g + 4

            def emit_av():
                for i in range(2):
                    kb = pair * 2 + i
                    for h in range(HPC):
                        nc.tensor.matmul(
                            av_ps[h][:, co[i]:QG], v_sb[:, kb, h, :],
                            p_t[h][:, i, co[i]:QG],
                            start=(kb == 0), stop=(kb == nkb - 1),
                            skip_group_check=True,
                        )
            return emit_av

        def make_norm(g, av_ps):
            def emit_norm():
                attn = []
                for h in range(HPC):
                    rs = apool.tile([1, QG], f32, tag="rs", name="rs")
                    nc.vector.tensor_copy(rs, av_ps[h][DH:DH + 1, :])
                    rec = apool.tile([1, QG], f32, tag="rec", name="rec")
                    nc.vector.reciprocal_approx_fast(rec, rs)
                    rb = apool.tile([DH, QG], f32, tag="rb", name="rb")
                    nc.gpsimd.partition_broadcast(rb, rec)
                    at = apool.tile([DH, QG], bf16, tag="at", name="at")
                    nc.vector.tensor_mul(at, av_ps[h][0:DH, :], rb)
                    attn.append(at)
                return attn
            return emit_norm

        def make_oproj(g, attn):
            q0 = g * QG

            def emit_oproj():
                for j in range(QG // 128):
                    o_ps = pmisc.tile([128, D], f32, tag="pm", name="o_ps")
                    nc.tensor.matmul(
                        o_ps, attn[0][:, j * 128:(j + 1) * 128], woa_sb,
                        start=True, stop=False, skip_group_check=True,
                    )
                    nc.tensor.matmul(
                        o_ps, attn[1][:, j * 128:(j + 1) * 128], wob_sb,
                        start=False, stop=True, skip_group_check=True,
                    )
                    ot = opool.tile([128, D], f32, tag="ot", name="ot")
                    nc.vector.tensor_copy(ot, o_ps)
                    nc.sync.dma_start(
                        out=out_part[q0 + j * 128:q0 + (j + 1) * 128, :], in_=ot
                    )
            return emit_oproj

        # ---- main interleaved loop --------------------------------------
        # Per g: project t-block g (k/q/v) from the prefetched raw tiles,
        # immediately issue the raw DMAs for block g+1 (ahead of the
        # normalize broadcasts so the GpSimd queue never delays them),
        # then run attention pairs for query group g. AV lags scores by
        # one pair; normalize+oproj of group g-1 are flushed inside group
        # g's first two pair iterations.
        rawt = emit_dma_block(0)
        cm_c = const.tile([128, 4, QG], bf16, name="cm_c")
        nc.gpsimd.dma_start(out=cm_c, in_=cmdram[:])
        cm_sb = cm_c
        bq_sb = const.tile([HPC * DH, 1], f32)
        nc.sync.dma_start(out=bq_sb, in_=bq2[:])
        bk_sb = const.tile([HPC * DH, 1], f32)
        nc.sync.dma_start(out=bk_sb, in_=bk2[:])
        bvr_sb = const.tile([1, HPC * DH], bf16)
        nc.gpsimd.dma_start(out=bvr_sb, in_=bvr[:])
        ones1_sb = const.tile([1, 128], bf16)
        nc.vector.memset(ones1_sb, 1.0)

        prev_av = None        # AV emission for the previous (g, pair)
        pend_norm = None      # normalize emission for the previous group
        pend_oproj_mk = None  # (g-1, attn) -> oproj emission
        for g in range(NQG):
            emit_proj(g, rawt)
            if g + 1 < NQG:
                rawt = emit_dma_block(g + 1)
            av_ps = [
                pmisc.tile([DH + 1, QG], f32, tag="pm", name="av_ps")
                for _ in range(HPC)
            ]
            for pair in range(2 * g + 2):
                p_t, co = emit_scores(g, pair)
                if prev_av is not None:
                    prev_av()
                if pend_norm is not None:
                    attn_prev = pend_norm()
                    pend_oproj_mk = make_oproj(g - 1, attn_prev)
                    pend_norm = None
                elif pend_oproj_mk is not None:
                    pend_oproj_mk()
                    pend_oproj_mk = None
                prev_av = make_av(g, pair, p_t, co, av_ps)
            prev_av()
            prev_av = None
            pend_norm = make_norm(g, av_ps)
        attn_last = pend_norm()
        make_oproj(NQG - 1, attn_last)()

    nc.compile()
    return nc


def _numpy_reference(query, key, value, mask, Wq, bq, Wk, bk, Wv, bv, Wo, bo):
    def split_heads(x):
        b, t, d = x.shape
        return x.reshape(b, t, H, DH).transpose(0, 2, 1, 3)

    q = split_heads(query @ Wq.T + bq)
    k = split_heads(key @ Wk.T + bk)
    v = split_heads(value @ Wv.T + bv)
    scale = 1.0 / np.sqrt(np.float32(DH))
    out = np.empty((B, H, T, DH), np.float32)
    for b in range(B):
        for h in range(H):
            s = (q[b, h] @ k[b, h].T) * scale
            s = np.where(mask[b] == 0, -np.inf, s)
            s = s - s.max(axis=-1, keepdims=True)
            p = np.exp(s)
            p /= p.sum(axis=-1, keepdims=True)
            out[b, h] = p @ v[b, h]
    out = out.transpose(0, 2, 1, 3).reshape(B, T, D)
    return out @ Wo.T + bo


def kernel(query, key, value, mask, Wq, bq, Wk, bk, Wv, bv, Wo, bo):
    global LAST_EXEC_TIME_NS, LAST_RESULTS
    query = np.asarray(query, np.float32)
    key = np.asarray(key, np.float32)
    value = np.asarray(value, np.float32)
    mask = np.asarray(mask)
    Wq, bq = np.asarray(Wq, np.float32), np.asarray(bq, np.float32)
    Wk, bk = np.asarray(Wk, np.float32), np.asarray(bk, np.float32)
    Wv, bv = np.asarray(Wv, np.float32), np.asarray(bv, np.float32)
    Wo, bo = np.asarray(Wo, np.float32), np.asarray(bo, np.float32)

    tril = np.tril(np.ones((T, T), mask.dtype))
    causal = all(np.array_equal(mask[b], tril) for b in range(B))
    if not causal:
        return _numpy_reference(
            query, key, value, mask, Wq, bq, Wk, bk, Wv, bv, Wo, bo
        ).astype(np.float32)

    r = np.arange(128, dtype=np.int64)[:, None]
    c = np.arange(QG, dtype=np.int64)[None, :]
    cmask = np.stack(
        [(c >= 128 * j + r).astype(np.float32) for j in range(4)], axis=1
    )  # (128, 4, QG)

    in_maps = []
    for core in range(NCORES):
        b = core // 4
        h0 = (core % 4) * HPC
        sl = slice(h0 * DH, (h0 + HPC) * DH)
        # wpack: wq|wk|wv (each [128, 4, 128] -> 512 cols), wo stacked
        # rows (A above B) [128, 512], cmask [128, 4, 512] -> 2048 cols.
        wq_r = np.ascontiguousarray(Wq[sl, :].T).reshape(CCH, 128, 128).transpose(1, 0, 2).reshape(128, 512)
        wk_r = np.ascontiguousarray(Wk[sl, :].T).reshape(CCH, 128, 128).transpose(1, 0, 2).reshape(128, 512)
        wv_r = np.ascontiguousarray(Wv[sl, :].T).reshape(CCH, 128, 128).transpose(1, 0, 2).reshape(128, 512)
        wo_r = np.zeros((128, 1024), np.float32)
        wo_r[0:DH, 0:512] = Wo[:, h0 * DH:(h0 + 1) * DH].T
        wo_r[0:DH, 512:1024] = Wo[:, (h0 + 1) * DH:(h0 + 2) * DH].T
        wpk = np.concatenate([wq_r, wk_r, wv_r, wo_r], axis=1).astype(np.float32)
        in_maps.append({
            "xqT": np.ascontiguousarray(query[b].T),
            "xkT": np.ascontiguousarray(key[b].T),
            "xvT": np.ascontiguousarray(value[b].T),
            "wpack": np.ascontiguousarray(wpk),
            "cmdram": cmask,
            "bq2": np.ascontiguousarray(bq[sl].reshape(HPC * DH, 1)),
            "bk2": np.ascontiguousarray(bk[sl].reshape(HPC * DH, 1)),
            "bvr": np.ascontiguousarray(bv[sl].reshape(1, HPC * DH)),
        })

    nc = _build_module()
    from concourse import bass_utils

    trace = os.environ.get("KERNEL_TRACE", "0") == "1"
    res = bass_utils.run_bass_kernel_spmd(
        nc, in_maps, core_ids=list(range(NCORES)), trace=trace
    )
    LAST_RESULTS = res
    LAST_EXEC_TIME_NS = res.exec_time_ns

    out = np.zeros((B, T, D), np.float32)
    for core in range(NCORES):
        out[core // 4] += np.asarray(res.results[core]["out_part"], np.float32)
    out += bo[None, None, :]
    return out


# revision 21
# speedup vs baseline: 1.6068x; 1.0562x over previous
"""Multi-head causal attention kernel for 8 Trainium2 NeuronCores.

Problem: B=2, T=4096, D=512, H=8 (DH=64) fp32 MHA with causal mask.

Sharding: 16 (b, h) pairs -> 2 heads per core (core c: b = c//4, heads
2*(c%4), 2*(c%4)+1). Each core projects q/k into feature-major (DH x T)
layout and v into t-major (T x DH) layout from host-pre-transposed
inputs, runs causal flash-style attention per head (scoresT on PE, exp
on ScalarE with the 1/sqrt(dh) scale folded in, per-block causal masks
on DVE, AV.T + rowsum accumulated in PSUM via a ones-column in the
stationary operand), normalizes via reciprocal + partition broadcast,
and applies the output projection for its 2 heads producing a partial
(T, D) output. The host sums the 4 partials per batch and adds the
output bias.

The projection work for t-block tb is interleaved with the attention
work for query-group g=tb so the PE stays dense (and HAM-warm) while
the raw input stream DMAs in; scores/exp/AV/normalize are software-
pipelined one step apart for the same reason.

The mask is verified host-side to be the causal tril; if not, a numpy
fallback computes the exact reference result.
"""

import os
import numpy as np

B, T, D, H = 2, 4096, 512, 8
DH = D // H          # 64
HPC = 2              # heads per core
NCORES = 8
QG = 512             # query-group width (matmul moving-operand size)
NQG = T // QG        # 8
NT = T // 128        # 32 key tiles
CCH = D // 128       # 4 contraction chunks for projections

# Weight packs (all cast to bf16 on load): wqk = wq | wk loads first so
# the k/q projections start as early as possible; wvo = wv | wo follows
# the first raw-input block. The wo region is 1024 cols with data only
# in partitions 0..63 ([woA | woB]) so both O-proj operands sit at
# partition base 0. The causal-mask patterns load separately.
WQK_COLS = 1024
WVO_COLS = 1536

LAST_EXEC_TIME_NS = None
LAST_RESULTS = None


def _build_module():
    import concourse.bacc as bacc
    import concourse.tile as tile
    from concourse import mybir
    from contextlib import ExitStack

    f32 = mybir.dt.float32
    bf16 = mybir.dt.bfloat16
    EXP = mybir.ActivationFunctionType.Exp

    nc = bacc.Bacc("TRN2", target_bir_lowering=False, debug=False)

    xqT = nc.dram_tensor("xqT", (D, T), f32, kind="ExternalInput")
    xkT = nc.dram_tensor("xkT", (D, T), f32, kind="ExternalInput")
    xvT = nc.dram_tensor("xvT", (D, T), f32, kind="ExternalInput")
    wqk = nc.dram_tensor("wqk", (128, WQK_COLS), f32, kind="ExternalInput")
    wvo = nc.dram_tensor("wvo", (128, WVO_COLS), f32, kind="ExternalInput")
    cmdram = nc.dram_tensor("cmdram", (128, 4, QG), f32, kind="ExternalInput")
    bq2 = nc.dram_tensor("bq2", (HPC * DH, 1), f32, kind="ExternalInput")
    bk2 = nc.dram_tensor("bk2", (HPC * DH, 1), f32, kind="ExternalInput")
    bvr = nc.dram_tensor("bvr", (1, HPC * DH), f32, kind="ExternalInput")
    out_part = nc.dram_tensor("out_part", (T, D), f32, kind="ExternalOutput")

    with tile.TileContext(nc) as tc, ExitStack() as ctx:
        const = ctx.enter_context(tc.tile_pool(name="const", bufs=1))
        resid = ctx.enter_context(tc.tile_pool(name="resid", bufs=1))
        raws = ctx.enter_context(tc.tile_pool(name="raws", bufs=24))
        ppool = ctx.enter_context(tc.tile_pool(name="ppool", bufs=4))
        apool = ctx.enter_context(tc.tile_pool(name="apool", bufs=4))
        opool = ctx.enter_context(tc.tile_pool(name="opool", bufs=3))
        pscore = ctx.enter_context(tc.tile_pool(name="pscore", bufs=2, space="PSUM"))
        pmisc = ctx.enter_context(tc.tile_pool(name="pmisc", bufs=4, space="PSUM"))

        # ---- constants: q/k weights first, the rest after the first
        # ---- raw-input block so the first matmuls start early ----
        wqk_sb = const.tile([128, WQK_COLS], bf16)
        nc.gpsimd.dma_start(out=wqk_sb, in_=wqk[:])
        wq_sb = wqk_sb[:, 0:512].rearrange("p (c m) -> p c m", c=CCH)
        wk_sb = wqk_sb[:, 512:1024].rearrange("p (c m) -> p c m", c=CCH)

        # ---- residents ----
        qT_sb = resid.tile([HPC * DH, T], bf16)   # feature-major q, 2 heads
        kT_sb = resid.tile([HPC * DH, T], bf16)   # feature-major k, 2 heads
        # t-major v, per key-tile: [vA(64) | 1] [vB(64) | 1]
        v_sb = resid.tile([128, NT, HPC, DH + 1], bf16)
        nc.vector.memset(v_sb[:, :, :, DH], 1.0)

        # ---- emission helpers -------------------------------------------
        def emit_dma_block(tb):
            """Issue the 12 raw-input cast-DMAs for t-block tb."""
            tiles = {}
            for key, src in (("k", xkT), ("q", xqT), ("v", xvT)):
                for cc in range(CCH):
                    raw = raws.tile([128, QG], bf16, tag="raw", name="raw")
                    nc.gpsimd.dma_start(
                        out=raw,
                        in_=src[cc * 128:(cc + 1) * 128, tb * QG:(tb + 1) * QG],
                    )
                    tiles[key, cc] = raw
            return tiles

        def emit_proj(tb, rawt):
            for key, wsb, bias_sb, dst in (
                ("k", wk_sb, bk_sb, kT_sb),
                ("q", wq_sb, bq_sb, qT_sb),
            ):
                ps = pmisc.tile([128, QG], f32, tag="pm", name="ps_proj")
                for cc in range(CCH):
                    nc.tensor.matmul(
                        ps, wsb[:, cc, :], rawt[key, cc],
                        start=(cc == 0), stop=(cc == CCH - 1),
                    )
                nc.vector.tensor_scalar_add(
                    dst[:, tb * QG:(tb + 1) * QG], ps, bias_sb
                )
            for j in range(QG // 128):
                tt = tb * 4 + j
                ps = pmisc.tile([128, HPC * DH], f32, tag="pm", name="ps_v")
                for cc in range(CCH):
                    nc.tensor.matmul(
                        ps, rawt["v", cc][:, j * 128:(j + 1) * 128], wv_sb[:, cc, :],
                        start=(cc == 0), stop=False, skip_group_check=True,
                    )
                nc.tensor.matmul(     # bias: out[t, d] += 1 * bv[d]
                    ps, ones1_sb, bvr_sb,
                    start=False, stop=True, skip_group_check=True,
                )
                nc.vector.tensor_copy(
                    v_sb[:, tt, :, 0:DH],
                    ps.rearrange("p (h d) -> p h d", h=HPC),
                )

        def emit_scores(g, pair):
            # Boundary key-blocks (kb >= 4g) only attend to query columns
            # >= 128*jj within the group; restrict work to those columns.
            q0 = g * QG
            s_ps, p_t = [], []
            co = [max(0, (pair * 2 + i - 4 * g) * 128) for i in range(2)]
            for h in range(HPC):
                s = pscore.tile([128, 2, QG], f32, tag="sc", name="s_ps")
                s_ps.append(s)
            for i in range(2):
                kb = pair * 2 + i
                for h in range(HPC):
                    nc.tensor.matmul(
                        s_ps[h][:, i, co[i]:QG],
                        kT_sb[h * DH:(h + 1) * DH, kb * 128:(kb + 1) * 128],
                        qT_sb[h * DH:(h + 1) * DH, q0 + co[i]:q0 + QG],
                        start=True, stop=True,
                        tile_position=(h * DH, 0),
                    )
            for h in range(HPC):
                p = ppool.tile([128, 2, QG], bf16, tag="p", name="p_t")
                if co[0] == 0 and co[1] <= 128:
                    nc.scalar.activation(p, s_ps[h], EXP, scale=0.125)
                else:
                    for i in range(2):
                        nc.scalar.activation(
                            p[:, i, co[i]:QG], s_ps[h][:, i, co[i]:QG],
                            EXP, scale=0.125,
                        )
                p_t.append(p)
            for i in range(2):
                jj = pair * 2 + i - 4 * g
                if jj >= 0:
                    for h in range(HPC):
                        nc.vector.tensor_mul(
                            p_t[h][:, i, co[i]:QG], p_t[h][:, i, co[i]:QG],
                            cm_sb[:, jj, co[i]:QG],
                        )
            return p_t, co

        def make_av(g, pair, p_t, co, av_ps):
            nkb = 4 * g + 4

            def emit_av():
                for i in range(2):
                    kb = pair * 2 + i
                    for h in range(HPC):
                        nc.tensor.matmul(
                            av_ps[h][:, co[i]:QG], v_sb[:, kb, h, :],
                            p_t[h][:, i, co[i]:QG],
                            start=(kb == 0), stop=(kb == nkb - 1),
                            skip_group_check=True,
                        )
            return emit_av

        def make_norm(g, av_ps):
            def emit_norm():
                attn = []
                for h in range(HPC):
                    rs = apool.tile([1, QG], f32, tag="rs", name="rs")
                    nc.vector.tensor_copy(rs, av_ps[h][DH:DH + 1, :])
                    rec = apool.tile([1, QG], f32, tag="rec", name="rec")
                    nc.vector.reciprocal_approx_fast(rec, rs)
                    rb = apool.tile([DH, QG], f32, tag="rb", name="rb")
                    nc.gpsimd.partition_broadcast(rb, rec)
                    at = apool.tile([DH, QG], bf16, tag="at", name="at")
                    nc.vector.tensor_mul(at, av_ps[h][0:DH, :], rb)
                    attn.append(at)
                return attn
            return emit_norm

        def make_oproj(g, attn):
            q0 = g * QG

            def emit_oproj():
                for j in range(QG // 128):
                    o_ps = pmisc.tile([128, D], f32, tag="pm", name="o_ps")
                    nc.tensor.matmul(
                        o_ps, attn[0][:, j * 128:(j + 1) * 128], woa_sb,
                        start=True, stop=False, skip_group_check=True,
                    )
                    nc.tensor.matmul(
                        o_ps, attn[1][:, j * 128:(j + 1) * 128], wob_sb,
                        start=False, stop=True, skip_group_check=True,
                    )
                    ot = opool.tile([128, D], f32, tag="ot", name="ot")
                    nc.vector.tensor_copy(ot, o_ps)
                    nc.sync.dma_start(
                        out=out_part[q0 + j * 128:q0 + (j + 1) * 128, :], in_=ot
                    )
            return emit_oproj

        # ---- main interleaved loop --------------------------------------
        # Per g: project t-block g (k/q/v) from the prefetched raw tiles,
        # immediately issue the raw DMAs for block g+1 (ahead of the
        # normalize broadcasts so the GpSimd queue never delays them),
        # then run attention pairs for query group g. AV lags scores by
        # one pair; normalize+oproj of group g-1 are flushed inside group
        # g's first two pair iterations.
        rawt = emit_dma_block(0)
        wvo_sb = const.tile([128, WVO_COLS], bf16)
        nc.gpsimd.dma_start(out=wvo_sb, in_=wvo[:])
        wv_sb = wvo_sb[:, 0:512].rearrange("p (c m) -> p c m", c=CCH)
        woa_sb = wvo_sb[0:DH, 512:1024]                      # [64, 512]
        wob_sb = wvo_sb[0:DH, 1024:1536]                     # [64, 512]
        cm_c = const.tile([128, 4, QG], bf16, name="cm_c")
        nc.gpsimd.dma_start(out=cm_c, in_=cmdram[:])
        cm_sb = cm_c
        bq_sb = const.tile([HPC * DH, 1], f32)
        nc.sync.dma_start(out=bq_sb, in_=bq2[:])
        bk_sb = const.tile([HPC * DH, 1], f32)
        nc.sync.dma_start(out=bk_sb, in_=bk2[:])
        bvr_sb = const.tile([1, HPC * DH], bf16)
        nc.gpsimd.dma_start(out=bvr_sb, in_=bvr[:])
        ones1_sb = const.tile([1, 128], bf16)
        nc.vector.memset(ones1_sb, 1.0)

        prev_av = None        # AV emission for the previous (g, pair)
        pend_norm = None      # normalize emission for the previous group
        pend_oproj_mk = None  # (g-1, attn) -> oproj emission
        for g in range(NQG):
            emit_proj(g, rawt)
            if g + 1 < NQG:
                rawt = emit_dma_block(g + 1)
            av_ps = [
                pmisc.tile([DH + 1, QG], f32, tag="pm", name="av_ps")
                for _ in range(HPC)
            ]
            for pair in range(2 * g + 2):
                p_t, co = emit_scores(g, pair)
                if prev_av is not None:
                    prev_av()
                if pend_norm is not None:
                    attn_prev = pend_norm()
                    pend_oproj_mk = make_oproj(g - 1, attn_prev)
                    pend_norm = None
                elif pend_oproj_mk is not None:
                    pend_oproj_mk()
                    pend_oproj_mk = None
                prev_av = make_av(g, pair, p_t, co, av_ps)
            # carry prev_av into the next group's first pair iteration so
            # the PE has AV work during that group's first exp.
            pend_norm = make_norm(g, av_ps)
        prev_av()
        attn_last = pend_norm()
        make_oproj(NQG - 1, attn_last)()

    nc.compile()
    return nc


def _numpy_reference(query, key, value, mask, Wq, bq, Wk, bk, Wv, bv, Wo, bo):
    def split_heads(x):
        b, t, d = x.shape
        return x.reshape(b, t, H, DH).transpose(0, 2, 1, 3)

    q = split_heads(query @ Wq.T + bq)
    k = split_heads(key @ Wk.T + bk)
    v = split_heads(value @ Wv.T + bv)
    scale = 1.0 / np.sqrt(np.float32(DH))
    out = np.empty((B, H, T, DH), np.float32)
    for b in range(B):
        for h in range(H):
            s = (q[b, h] @ k[b, h].T) * scale
            s = np.where(mask[b] == 0, -np.inf, s)
            s = s - s.max(axis=-1, keepdims=True)
            p = np.exp(s)
            p /= p.sum(axis=-1, keepdims=True)
            out[b, h] = p @ v[b, h]
    out = out.transpose(0, 2, 1, 3).reshape(B, T, D)
    return out @ Wo.T + bo


def kernel(query, key, value, mask, Wq, bq, Wk, bk, Wv, bv, Wo, bo):
    global LAST_EXEC_TIME_NS, LAST_RESULTS
    query = np.asarray(query, np.float32)
    key = np.asarray(key, np.float32)
    value = np.asarray(value, np.float32)
    mask = np.asarray(mask)
    Wq, bq = np.asarray(Wq, np.float32), np.asarray(bq, np.float32)
    Wk, bk = np.asarray(Wk, np.float32), np.asarray(bk, np.float32)
    Wv, bv = np.asarray(Wv, np.float32), np.asarray(bv, np.float32)
    Wo, bo = np.asarray(Wo, np.float32), np.asarray(bo, np.float32)

    tril = np.tril(np.ones((T, T), mask.dtype))
    causal = all(np.array_equal(mask[b], tril) for b in range(B))
    if not causal:
        return _numpy_reference(
            query, key, value, mask, Wq, bq, Wk, bk, Wv, bv, Wo, bo
        ).astype(np.float32)

    r = np.arange(128, dtype=np.int64)[:, None]
    c = np.arange(QG, dtype=np.int64)[None, :]
    cmask = np.stack(
        [(c >= 128 * j + r).astype(np.float32) for j in range(4)], axis=1
    )  # (128, 4, QG)

    in_maps = []
    for core in range(NCORES):
        b = core // 4
        h0 = (core % 4) * HPC
        sl = slice(h0 * DH, (h0 + HPC) * DH)
        # wpack: wq|wk|wv (each [128, 4, 128] -> 512 cols), wo stacked
        # rows (A above B) [128, 512], cmask [128, 4, 512] -> 2048 cols.
        wq_r = np.ascontiguousarray(Wq[sl, :].T).reshape(CCH, 128, 128).transpose(1, 0, 2).reshape(128, 512)
        wk_r = np.ascontiguousarray(Wk[sl, :].T).reshape(CCH, 128, 128).transpose(1, 0, 2).reshape(128, 512)
        wv_r = np.ascontiguousarray(Wv[sl, :].T).reshape(CCH, 128, 128).transpose(1, 0, 2).reshape(128, 512)
        wo_r = np.zeros((128, 1024), np.float32)
        wo_r[0:DH, 0:512] = Wo[:, h0 * DH:(h0 + 1) * DH].T
        wo_r[0:DH, 512:1024] = Wo[:, (h0 + 1) * DH:(h0 + 2) * DH].T
        in_maps.append({
            "xqT": np.ascontiguousarray(query[b].T),
            "xkT": np.ascontiguousarray(key[b].T),
            "xvT": np.ascontiguousarray(value[b].T),
            "wqk": np.ascontiguousarray(np.concatenate([wq_r, wk_r], axis=1)).astype(np.float32),
            "wvo": np.ascontiguousarray(np.concatenate([wv_r, wo_r], axis=1)).astype(np.float32),
            "cmdram": cmask,
            "bq2": np.ascontiguousarray(bq[sl].reshape(HPC * DH, 1)),
            "bk2": np.ascontiguousarray(bk[sl].reshape(HPC * DH, 1)),
            "bvr": np.ascontiguousarray(bv[sl].reshape(1, HPC * DH)),
        })

    nc = _build_module()
    from concourse import bass_utils

    trace = os.environ.get("KERNEL_TRACE", "0") == "1"
    res = bass_utils.run_bass_kernel_spmd(
        nc, in_maps, core_ids=list(range(NCORES)), trace=trace
    )
    LAST_RESULTS = res
    LAST_EXEC_TIME_NS = res.exec_time_ns

    out = np.zeros((B, T, D), np.float32)
    for core in range(NCORES):
        out[core // 4] += np.asarray(res.results[core]["out_part"], np.float32)
    out += bo[None, None, :]
    return out
